# revision 14
# baseline (speedup 1.0000x reference)
"""Trainium2 Bass kernel for nn_Attention_44994077393310.

Multi-head attention (B=8, N=2048, C=768, H=4, Dh=192) with input projections,
softmax attention, and output projection with bias.

Sharding: pure data-parallel over the batch dim - each of the 8 NeuronCores
computes one batch element end-to-end (weights replicated). No collectives.

v2 design notes (vs the 643us baseline):
  - The baseline ran ~45% of the kernel at the PE's cold clock (HAM K=4/8,
    1.2 GHz): the scores->exp->AV inner loop had ~35% PE micro-idle per step,
    which kept re-throttling the clock gate. This version restructures the
    attention loop around HEAD PAIRS so the PE stream is dense:
      * the two K=64 score remainder matmuls of a head pair sit at array row
        groups 0-1 / 2-3 (base partitions 0/64) and are issued back-to-back,
        so they run CONCURRENTLY (row-tiled packing);
      * one 2-bank PSUM tile [128,1024] holds the pair's scores, and a single
        ACTIVATE exps both heads (1147ns vs 2x720ns), keeping ScalarE off the
        critical path;
      * softmax denominators: 1/rowsum via reciprocal_approx_fast (DVE custom
        op, ~0.7us vs 3.35us for the iterative divide), with the PSUM->SBUF
        bounce the op requires;
      * U is evacuated UNNORMALIZED with plain DVE copies right at the pair
        boundary (frees the 4 u PSUM banks for the next pair within ~2us);
        the 1/r normalization happens later as in-place DVE multiplies
        against a PE rank-1 broadcast, entirely off the critical path.
  - q is projected per chunk at the PREVIOUS chunk boundary (PE-dense work
    that covers the finalize chains); q/k/v/final projections and the rank-1
    broadcasts all share the score PSUM pool, so PSUM is exactly 8 banks.
  - the output projection folds the bias in as a K=1 ones-row matmul and
    evacuates y on ScalarE (idle at boundaries), keeping DVE under budget.
"""

import numpy as np

B = 8
N = 2048
C = 768
H = 4
DH = 192
SCALE = DH ** -0.5

NCHUNKS = 4                # chunks of 512 over the sequence
CHUNK = N // NCHUNKS       # 512
CC = C // 128              # 6 channel chunks
KT = N // 128              # 16 k-tiles

_BUILT = None


def _dest_of(cp):
    h, dd = divmod(cp, DH)
    if dd < 128:
        return ("a", h, dd)
    return ("b", h // 2, (h % 2) * 64 + (dd - 128))


def _jc_segments(jc):
    """Merged PSUM->head-major copy segments for projection j-chunk jc."""
    segs = []
    for p0 in range(0, 128, 64):
        kind, idx, dlo = _dest_of(128 * jc + p0)
        if segs and segs[-1][2] == kind and segs[-1][3] == idx and \
                segs[-1][4] + (segs[-1][1] - segs[-1][0]) == dlo:
            segs[-1] = (segs[-1][0], p0 + 64, kind, idx, segs[-1][4])
        else:
            segs.append((p0, p0 + 64, kind, idx, dlo))
    return segs


def _build():
    from contextlib import ExitStack

    import concourse.mybir as mybir
    import concourse.tile as tile
    from concourse import bacc

    F32 = mybir.dt.float32
    MMD = mybir.dt.float32r
    AF = mybir.ActivationFunctionType

    nc = bacc.Bacc("TRN2", target_bir_lowering=False, debug=False)
    qt_d = nc.dram_tensor("qT", [C, N], F32, kind="ExternalInput").ap()
    kt_d = nc.dram_tensor("kT", [C, N], F32, kind="ExternalInput").ap()
    vt_d = nc.dram_tensor("vT", [C, N], F32, kind="ExternalInput").ap()
    wqt_d = nc.dram_tensor("WqT", [C, C], F32, kind="ExternalInput").ap()
    wkt_d = nc.dram_tensor("WkT", [C, C], F32, kind="ExternalInput").ap()
    wvt_d = nc.dram_tensor("WvT", [C, C], F32, kind="ExternalInput").ap()
    wpt_d = nc.dram_tensor("WpT", [C, C], F32, kind="ExternalInput").ap()
    bp_d = nc.dram_tensor("bp", [C], F32, kind="ExternalInput").ap()
    y_d = nc.dram_tensor("y", [N, C], F32, kind="ExternalOutput").ap()

    with tile.TileContext(nc) as tc, ExitStack() as ctx:
        const = ctx.enter_context(tc.tile_pool(name="const", bufs=1))
        wqp = ctx.enter_context(tc.tile_pool(name="wqp", bufs=1))
        khp = ctx.enter_context(tc.tile_pool(name="khp", bufs=1))
        vhp = ctx.enter_context(tc.tile_pool(name="vhp", bufs=1))
        psS = ctx.enter_context(tc.tile_pool(name="psS", bufs=2, space="PSUM"))
        psU = ctx.enter_context(tc.tile_pool(name="psU", bufs=1, space="PSUM"))

        ones_col_f32 = const.tile([128, H], F32, tag="ones_col", name="ones_col")
        nc.vector.memset(ones_col_f32[:], 1.0)
        ones_row_f32 = const.tile([1, 128], F32, tag="ones_row_f", name="ones_row_f")
        nc.vector.memset(ones_row_f32[:], 1.0)
        ones_row = const.tile([1, 128], MMD, tag="ones_row", name="ones_row")
        nc.vector.tensor_copy(ones_row[:], ones_row_f32[:])

        # PE warm-up: dependency-free matmuls so the HAM clock gate opens
        # while the first DMAs stream in.
        warm_w_f = const.tile([128, 128], F32, tag="warm_w_f", name="warm_w_f")
        nc.vector.memset(warm_w_f[:], 0.5)
        warm_w = const.tile([128, 128], MMD, tag="warm_w", name="warm_w")
        nc.vector.tensor_copy(warm_w[:], warm_w_f[:])
        warm_x = const.tile([128, 512], MMD, tag="warm_x", name="warm_x")
        for i in range(4):
            nc.vector.tensor_copy(warm_x[:, i * 128:(i + 1) * 128], warm_w_f[:])
        for r in range(14):
            wp = psS.tile([128, 1024], F32, tag="S", name="S")
            nc.tensor.matmul(wp[:, 0:512], warm_w[:], warm_x[:],
                             start=True, stop=True)
            nc.tensor.matmul(wp[:, 512:1024], warm_w[:], warm_x[:],
                             start=True, stop=True)

        # ---- persistent weights ----
        WqT = wqp.tile([128, CC, C], MMD, tag="wqt", name="wqt")
        WpT_a = wqp.tile([128, H, C], MMD, tag="wpa", name="wpa")
        WpT_b = [wqp.tile([128, C], MMD, tag=f"wpb{g}", name=f"wpb{g}")
                 for g in range(2)]
        bp_row = wqp.tile([1, C], MMD, tag="bp_row", name="bp_row")

        khT_a = [khp.tile([128, N], MMD, tag=f"kha{h}", name=f"kha{h}")
                 for h in range(H)]
        khT_b = [khp.tile([128, N], MMD, tag=f"khb{g}", name=f"khb{g}")
                 for g in range(2)]
        vh = [vhp.tile([128, H, DH + 1], MMD, tag=f"vh{nt}", name=f"vh{nt}")
              for nt in range(KT)]

        def load_wT_grouped(dest, w_dram):
            # dest[p, cc, j] = W.T[cc*128+p, j]
            nc.gpsimd.dma_start(
                dest[:],
                w_dram.rearrange("(cc p) j -> p cc j", p=128))

        def seg_dest(kind, idx, dlo, dhi, a_tiles, b_tiles, col_lo, col_hi):
            t = a_tiles[idx] if kind == "a" else b_tiles[idx]
            return t[dlo:dhi, col_lo:col_hi]

        # ---- phase 1: stage k, then v ------------------------------------
        # One gpsimd cast-DMA queue; ordered WkT, k0..k3, WvT, v0..v3, WqT,
        # qT0, WpT, bias so the first projection starts ~11us in (covered by
        # the warm-up) and nothing later stalls on the queue.
        with tc.tile_pool(name="stg", bufs=1) as stg:
            WkT = stg.tile([128, CC, C], MMD, tag="wkt", name="wkt")
            WvT = stg.tile([128, CC, C], MMD, tag="wvt", name="wvt")
            load_wT_grouped(WkT, wkt_d)

            for ch in range(NCHUNKS):
                n0 = ch * CHUNK
                kTt = stg.tile([128, CC, CHUNK], MMD, tag="xT", name="kTt",
                               bufs=2)
                nc.gpsimd.dma_start(
                    kTt[:],
                    kt_d[:, n0:n0 + CHUNK].rearrange("(cc p) n -> p cc n", p=128))
                if ch == 2:
                    load_wT_grouped(WvT, wvt_d)
                for jc0 in range(0, CC, 2):
                    ps = psS.tile([128, 1024], F32, tag="S", name="S")
                    for cc in range(CC):
                        for i in range(2):
                            nc.tensor.matmul(
                                ps[:, i * 512:(i + 1) * 512],
                                WkT[:, cc, (jc0 + i) * 128:(jc0 + i + 1) * 128],
                                kTt[:, cc, :], start=(cc == 0),
                                stop=(cc == CC - 1))
                    for i in range(2):
                        for (plo, phi, kind, idx, dlo) in _jc_segments(jc0 + i):
                            nc.vector.tensor_copy(
                                seg_dest(kind, idx, dlo, dlo + (phi - plo),
                                         khT_a, khT_b, n0, n0 + CHUNK),
                                ps[plo:phi, i * 512:(i + 1) * 512])

            for ch in range(NCHUNKS):
                n0 = ch * CHUNK
                vTt = stg.tile([128, CC, CHUNK], MMD, tag="xT", name="vTt",
                               bufs=2)
                nc.gpsimd.dma_start(
                    vTt[:],
                    vt_d[:, n0:n0 + CHUNK].rearrange("(cc p) n -> p cc n", p=128))
                if ch == 0:
                    load_wT_grouped(WqT, wqt_d)
                elif ch == 1:
                    # wpt_d is host-packed head-major: rows 0..511 = per-head
                    # dd 0..127 (h-major), rows 512..639 / 640..767 = the
                    # packed b-tiles (dd 128..191 of heads 0,1 / 2,3).
                    nc.gpsimd.dma_start(
                        WpT_a[:],
                        wpt_d[0:512, :].rearrange("(h p) j -> p h j", p=128))
                    for g in range(2):
                        nc.gpsimd.dma_start(
                            WpT_b[g][:],
                            wpt_d[512 + g * 128:512 + (g + 1) * 128, :])
                    nc.gpsimd.dma_start(bp_row[:], bp_d[None, :])
                for ntl in range(4):
                    nt = ch * 4 + ntl
                    ps = psS.tile([128, 1024], F32, tag="S", name="S")
                    for cc in range(CC):
                        for jg in range(2):
                            nc.tensor.matmul(
                                ps[:, jg * 512:jg * 512 + 384],
                                vTt[:, cc, ntl * 128:(ntl + 1) * 128],
                                WvT[:, cc, jg * 384:(jg + 1) * 384],
                                start=(cc == 0), stop=(cc == CC - 1))
                    for jg in range(2):
                        nc.vector.tensor_copy(
                            vh[nt][:, 2 * jg:2 * jg + 2, 0:DH],
                            ps[:, jg * 512:jg * 512 + 384].rearrange(
                                "p (h d) -> p h d", h=2))
                    nc.vector.tensor_copy(
                        vh[nt][:, :, DH:DH + 1],
                        ones_col_f32[:].rearrange("p (h o) -> p h o", h=H))

        # ---- phase 2 pools ----------------------------------------------
        qtp = ctx.enter_context(tc.tile_pool(name="qtp", bufs=1))
        qhp = ctx.enter_context(tc.tile_pool(name="qhp", bufs=1))
        esp = ctx.enter_context(tc.tile_pool(name="esp", bufs=2))
        xop = ctx.enter_context(tc.tile_pool(name="xop", bufs=1))
        scp = ctx.enter_context(tc.tile_pool(name="scp", bufs=1))
        bcp = ctx.enter_context(tc.tile_pool(name="bcp", bufs=1))
        yp = ctx.enter_context(tc.tile_pool(name="yp", bufs=2))

        def q_load(qc):
            n0 = qc * CHUNK
            qTt = qtp.tile([128, CC, CHUNK], MMD, tag="qTt", name="qTt")
            nc.gpsimd.dma_start(
                qTt[:],
                qt_d[:, n0:n0 + CHUNK].rearrange("(cc p) n -> p cc n", p=128))
            return qTt


        xT_a = [xop.tile([128, CHUNK], MMD, tag=f"xta{h}", name=f"xta{h}")
                for h in range(H)]
        xT_b = [xop.tile([128, CHUNK], MMD, tag=f"xtb{g}", name=f"xtb{g}")
                for g in range(2)]

        def q_proj(qTt, interleave=None):
            """Project one q chunk. `interleave(j)` is called between
            jc-groups to slot independent PE/DVE work into the stream."""
            qhT_a = [qhp.tile([128, CHUNK], MMD, tag=f"qha{h}", name=f"qha{h}")
                     for h in range(H)]
            qhT_b = [qhp.tile([128, CHUNK], MMD, tag=f"qhb{g}", name=f"qhb{g}")
                     for g in range(2)]
            for j, jc0 in enumerate(range(0, CC, 2)):
                ps = psS.tile([128, 1024], F32, tag="S", name="S")
                for cc in range(CC):
                    for i in range(2):
                        nc.tensor.matmul(
                            ps[:, i * 512:(i + 1) * 512],
                            WqT[:, cc, (jc0 + i) * 128:(jc0 + i + 1) * 128],
                            qTt[:, cc, :], start=(cc == 0), stop=(cc == CC - 1))
                for i in range(2):
                    for (plo, phi, kind, idx, dlo) in _jc_segments(jc0 + i):
                        nc.vector.tensor_copy(
                            seg_dest(kind, idx, dlo, dlo + (phi - plo),
                                     qhT_a, qhT_b, 0, CHUNK),
                            ps[plo:phi, i * 512:(i + 1) * 512])
                if interleave is not None:
                    interleave(j)
            return qhT_a, qhT_b

        def pair_attention(g, qhT_a, qhT_b, qc_end=False, interleave=None):
            """One head pair (heads 2g, 2g+1) over all 16 k-tiles.
            Scores run TWO k-tiles ahead so the exp ACTIVATE has a full
            iteration of slack - av(kt) never waits on ScalarE even though
            the Tile scheduler orders av before scores in the PE queue.
            Returns the two heads' f32r reciprocal rows."""
            h0, h1 = 2 * g, 2 * g + 1
            uA0 = psU.tile([128, 512], F32, tag="uA0", name="uA0")
            uB0 = psU.tile([65, 512], F32, tag="uB0", name="uB0")
            uA1 = psU.tile([128, 512], F32, tag="uA1", name="uA1")
            uB1 = psU.tile([65, 512], F32, tag="uB1", name="uB1")
            es_t = [None] * KT

            def scores(kt):
                S = psS.tile([128, 1024], F32, tag="S", name="S")
                nc.tensor.matmul(
                    S[:, 0:512], khT_a[h0][:, kt * 128:(kt + 1) * 128],
                    qhT_a[h0][:], start=True, stop=False)
                nc.tensor.matmul(
                    S[:, 512:1024], khT_a[h1][:, kt * 128:(kt + 1) * 128],
                    qhT_a[h1][:], start=True, stop=False)
                # the two K=64 remainders sit at array row groups 0-1 / 2-3
                # (base partitions 0 / 64) and are adjacent in the stream, so
                # they execute concurrently.
                nc.tensor.matmul(
                    S[:, 0:512], khT_b[g][0:64, kt * 128:(kt + 1) * 128],
                    qhT_b[g][0:64, :], start=False, stop=True)
                nc.tensor.matmul(
                    S[:, 512:1024], khT_b[g][64:128, kt * 128:(kt + 1) * 128],
                    qhT_b[g][64:128, :], start=False, stop=True)
                es = esp.tile([128, 1024], MMD, tag="es", name="es")
                nc.scalar.activation(es[:], S[:], AF.Exp, scale=SCALE)
                es_t[kt] = es

            def av(kt):
                es = es_t[kt]
                st, sp = kt == 0, kt == KT - 1
                nc.tensor.matmul(uA0[:], vh[kt][:, h0, 0:128], es[:, 0:512],
                                 start=st, stop=sp)
                nc.tensor.matmul(uB0[:], vh[kt][:, h0, 128:DH + 1],
                                 es[:, 0:512], start=st, stop=sp)
                nc.tensor.matmul(uA1[:], vh[kt][:, h1, 0:128],
                                 es[:, 512:1024], start=st, stop=sp)
                nc.tensor.matmul(uB1[:], vh[kt][:, h1, 128:DH + 1],
                                 es[:, 512:1024], start=st, stop=sp)
                es_t[kt] = None

            scores(0)
            scores(1)
            for kt in range(KT):
                if kt + 2 < KT:
                    scores(kt + 2)
                av(kt)
                if interleave is not None and kt in interleave:
                    interleave[kt]()

            # boundary: evacuate U unnormalized (frees the u banks), bounce
            # the rowsum rows to SBUF, reciprocal.  Mid-qc the next pair
            # reuses the u banks within ~2 iterations, so the big copies go
            # first (uA0 frees first); at the qc end the recip chain gates
            # the bc matmuls, so the rowsum rows go first.
            def rchain(h, uB):
                row = scp.tile([1, 512], F32, tag="row", name="row")
                nc.vector.tensor_copy(row[:], uB[64:65, :])
                rt = scp.tile([1, 512], F32, tag="rt", name="rt")
                nc.vector.reciprocal_approx_fast(rt[:], row[:])
                rtr = scp.tile([1, 512], MMD, tag=f"rtr{h}", name=f"rtr{h}")
                nc.vector.tensor_copy(rtr[:], rt[:])
                return rtr

            rtrs = []
            if qc_end:
                for (h, uA, uB, blo) in ((h0, uA0, uB0, 0), (h1, uA1, uB1, 64)):
                    rtrs.append(rchain(h, uB))
                for (h, uA, uB, blo) in ((h0, uA0, uB0, 0), (h1, uA1, uB1, 64)):
                    nc.vector.tensor_copy(xT_a[h][:], uA[:])
                    nc.vector.tensor_copy(xT_b[g][blo:blo + 64, :], uB[0:64, :])
            else:
                for (h, uA, uB, blo) in ((h0, uA0, uB0, 0), (h1, uA1, uB1, 64)):
                    nc.vector.tensor_copy(xT_a[h][:], uA[:])
                    nc.vector.tensor_copy(xT_b[g][blo:blo + 64, :], uB[0:64, :])
                    rtrs.append(rchain(h, uB))
            return rtrs

        def bc_normalize(g, rtrs):
            """Rank-1 broadcast of 1/r for heads 2g,2g+1 + in-place
            normalization of their xT tiles. Emitted at the qc boundary,
            off the critical path."""
            h0, h1 = 2 * g, 2 * g + 1
            ps = psS.tile([128, 1024], F32, tag="S", name="S")
            nc.tensor.matmul(ps[:, 0:512], ones_row[:], rtrs[0][:],
                             start=True, stop=True)
            nc.tensor.matmul(ps[:, 512:1024], ones_row[:], rtrs[1][:],
                             start=True, stop=True)
            bc = bcp.tile([128, 1024], MMD, tag="bc", name="bc")
            nc.vector.tensor_copy(bc[:], ps[:])
            nc.vector.tensor_mul(xT_a[h0][:], xT_a[h0][:], bc[:, 0:512])
            nc.vector.tensor_mul(xT_b[g][0:64, :], xT_b[g][0:64, :],
                                 bc[0:64, 0:512])
            nc.vector.tensor_mul(xT_a[h1][:], xT_a[h1][:], bc[:, 512:1024])
            nc.vector.tensor_mul(xT_b[g][64:128, :], xT_b[g][64:128, :],
                                 bc[64:128, 512:1024])

        def final_proj(qc):
            n0 = qc * CHUNK
            for ntl in range(4):
                ps = psS.tile([128, 1024], F32, tag="S", name="S")
                # bias as a K=1 ones-row matmul opening both accumulation
                # groups (j 0:512 in bank 0, j 512:768 in bank 1)
                nc.tensor.matmul(ps[:, 0:512], ones_row[:], bp_row[:, 0:512],
                                 start=True, stop=False)
                nc.tensor.matmul(ps[:, 512:768], ones_row[:], bp_row[:, 512:C],
                                 start=True, stop=False)
                for h in range(H):
                    blo = (h % 2) * 64
                    last = h == H - 1
                    nc.tensor.matmul(
                        ps[:, 0:512],
                        xT_a[h][:, ntl * 128:(ntl + 1) * 128],
                        WpT_a[:, h, 0:512], start=False, stop=False)
                    nc.tensor.matmul(
                        ps[:, 512:768],
                        xT_a[h][:, ntl * 128:(ntl + 1) * 128],
                        WpT_a[:, h, 512:C], start=False, stop=False)
                    nc.tensor.matmul(
                        ps[:, 0:512],
                        xT_b[h // 2][blo:blo + 64, ntl * 128:(ntl + 1) * 128],
                        WpT_b[h // 2][blo:blo + 64, 0:512],
                        start=False, stop=last)
                    nc.tensor.matmul(
                        ps[:, 512:768],
                        xT_b[h // 2][blo:blo + 64, ntl * 128:(ntl + 1) * 128],
                        WpT_b[h // 2][blo:blo + 64, 512:C],
                        start=False, stop=last)
                ysb = yp.tile([128, C], F32, tag="y", name="y")
                nc.scalar.copy(ysb[:], ps[:, 0:C])
                nc.sync.dma_start(
                    y_d[n0 + ntl * 128:n0 + (ntl + 1) * 128, :], ysb[:])

        # ---- phase 2: per q-chunk attention + projections ------------------
        qh = q_proj(q_load(0))
        for qc in range(NCHUNKS):
            last_qc = qc + 1 == NCHUNKS
            if not last_qc:
                qt_next = q_load(qc + 1)
            rtrs0 = pair_attention(0, *qh, qc_end=False)
            if last_qc:
                # no q_proj to cover the boundary: pre-normalize pair 0
                # mid-pair-1 (one S-rotation hiccup, but final_proj's h0/h1
                # matmuls become ready work at the boundary)
                rtrs1 = pair_attention(
                    1, *qh, qc_end=True,
                    interleave={4: (lambda _r0=rtrs0: bc_normalize(0, _r0))})
                bc_normalize(1, rtrs1)
            else:
                rtrs1 = pair_attention(1, *qh, qc_end=True)
                # qc boundary: the next chunk's q projection leads the PE
                # queue (independent work); the bc chains slot in after
                # groups 0/1, by which time the DVE recip chains are done.
                def inter(j, _r0=rtrs0, _r1=rtrs1):
                    if j == 0:
                        bc_normalize(0, _r0)
                    elif j == 1:
                        bc_normalize(1, _r1)
                qh = q_proj(qt_next, interleave=inter)
            final_proj(qc)

    nc.compile()
    return nc


def _get_built():
    global _BUILT
    if _BUILT is None:
        _BUILT = _build()
    return _BUILT


def run(inputs, trace=False, **kw):
    """Run on all 8 cores; returns (y [B,N,C] float32, BassKernelResults)."""
    from concourse.bass_utils import run_bass_kernel_spmd

    nc = _get_built()
    f32 = np.float32
    wpt = np.asarray(inputs["Wp"], f32).T  # [c', j]
    wpt_packed = np.concatenate(
        [wpt[h * DH:h * DH + 128] for h in range(H)]
        + [wpt[h * DH + 128:(h + 1) * DH] for h in range(H)])
    shared = {
        "WqT": np.ascontiguousarray(np.asarray(inputs["Wq"], f32).T),
        "WkT": np.ascontiguousarray(np.asarray(inputs["Wk"], f32).T),
        "WvT": np.ascontiguousarray(np.asarray(inputs["Wv"], f32).T),
        "WpT": np.ascontiguousarray(wpt_packed),
        "bp": np.ascontiguousarray(np.asarray(inputs["bp"], f32)),
    }
    q = np.asarray(inputs["q"], f32)
    k = np.asarray(inputs["k"], f32)
    v = np.asarray(inputs["v"], f32)
    in_maps = []
    for b in range(B):
        m = dict(shared)
        m["qT"] = np.ascontiguousarray(q[b].T)
        m["kT"] = np.ascontiguousarray(k[b].T)
        m["vT"] = np.ascontiguousarray(v[b].T)
        in_maps.append(m)
    res = run_bass_kernel_spmd(nc, in_maps, list(range(B)), trace=trace, **kw)
    y = np.stack([res.results[b]["y"] for b in range(B)]).astype(np.float32)
    return y, res


def kernel(q, k, v, Wq, Wk, Wv, Wp, bp):
    y, _ = run({"q": q, "k": k, "v": v, "Wq": Wq, "Wk": Wk, "Wv": Wv,
                "Wp": Wp, "bp": bp})
    return y


# revision 15
# speedup vs baseline: 1.0170x; 1.0170x over previous
"""Trainium2 Bass kernel for nn_Attention_44994077393310.

Multi-head attention (B=8, N=2048, C=768, H=4, Dh=192) with input projections,
softmax attention, and output projection with bias.

Sharding: pure data-parallel over the batch dim - each of the 8 NeuronCores
computes one batch element end-to-end (weights replicated). No collectives.

v2 design notes (vs the 643us baseline):
  - The baseline ran ~45% of the kernel at the PE's cold clock (HAM K=4/8,
    1.2 GHz): the scores->exp->AV inner loop had ~35% PE micro-idle per step,
    which kept re-throttling the clock gate. This version restructures the
    attention loop around HEAD PAIRS so the PE stream is dense:
      * the two K=64 score remainder matmuls of a head pair sit at array row
        groups 0-1 / 2-3 (base partitions 0/64) and are issued back-to-back,
        so they run CONCURRENTLY (row-tiled packing);
      * one 2-bank PSUM tile [128,1024] holds the pair's scores, and a single
        ACTIVATE exps both heads (1147ns vs 2x720ns), keeping ScalarE off the
        critical path;
      * softmax denominators: 1/rowsum via reciprocal_approx_fast (DVE custom
        op, ~0.7us vs 3.35us for the iterative divide), with the PSUM->SBUF
        bounce the op requires;
      * U is evacuated UNNORMALIZED with plain DVE copies right at the pair
        boundary (frees the 4 u PSUM banks for the next pair within ~2us);
        the 1/r normalization happens later as in-place DVE multiplies
        against a PE rank-1 broadcast, entirely off the critical path.
  - q is projected per chunk at the PREVIOUS chunk boundary (PE-dense work
    that covers the finalize chains); q/k/v/final projections and the rank-1
    broadcasts all share the score PSUM pool, so PSUM is exactly 8 banks.
  - the output projection folds the bias in as a K=1 ones-row matmul and
    evacuates y on ScalarE (idle at boundaries), keeping DVE under budget.
"""

import numpy as np

B = 8
N = 2048
C = 768
H = 4
DH = 192
SCALE = DH ** -0.5

NCHUNKS = 4                # chunks of 512 over the sequence
CHUNK = N // NCHUNKS       # 512
CC = C // 128              # 6 channel chunks
KT = N // 128              # 16 k-tiles

_BUILT = None


def _dest_of(cp):
    h, dd = divmod(cp, DH)
    if dd < 128:
        return ("a", h, dd)
    return ("b", h // 2, (h % 2) * 64 + (dd - 128))


def _jc_segments(jc):
    """Merged PSUM->head-major copy segments for projection j-chunk jc."""
    segs = []
    for p0 in range(0, 128, 64):
        kind, idx, dlo = _dest_of(128 * jc + p0)
        if segs and segs[-1][2] == kind and segs[-1][3] == idx and \
                segs[-1][4] + (segs[-1][1] - segs[-1][0]) == dlo:
            segs[-1] = (segs[-1][0], p0 + 64, kind, idx, segs[-1][4])
        else:
            segs.append((p0, p0 + 64, kind, idx, dlo))
    return segs


def _build():
    from contextlib import ExitStack

    import concourse.mybir as mybir
    import concourse.tile as tile
    from concourse import bacc

    F32 = mybir.dt.float32
    MMD = mybir.dt.float32r
    AF = mybir.ActivationFunctionType

    nc = bacc.Bacc("TRN2", target_bir_lowering=False, debug=False)
    qt_d = nc.dram_tensor("qT", [C, N], F32, kind="ExternalInput").ap()
    kt_d = nc.dram_tensor("kT", [C, N], F32, kind="ExternalInput").ap()
    vt_d = nc.dram_tensor("vT", [C, N], F32, kind="ExternalInput").ap()
    wqt_d = nc.dram_tensor("WqT", [C, C], F32, kind="ExternalInput").ap()
    wkt_d = nc.dram_tensor("WkT", [C, C], F32, kind="ExternalInput").ap()
    wvt_d = nc.dram_tensor("WvT", [C, C], F32, kind="ExternalInput").ap()
    wpt_d = nc.dram_tensor("WpT", [C, C], F32, kind="ExternalInput").ap()
    bp_d = nc.dram_tensor("bp", [C], F32, kind="ExternalInput").ap()
    y_d = nc.dram_tensor("y", [N, C], F32, kind="ExternalOutput").ap()

    with tile.TileContext(nc) as tc, ExitStack() as ctx:
        const = ctx.enter_context(tc.tile_pool(name="const", bufs=1))
        wqp = ctx.enter_context(tc.tile_pool(name="wqp", bufs=1))
        khp = ctx.enter_context(tc.tile_pool(name="khp", bufs=1))
        vhp = ctx.enter_context(tc.tile_pool(name="vhp", bufs=1))
        psS = ctx.enter_context(tc.tile_pool(name="psS", bufs=2, space="PSUM"))
        psU = ctx.enter_context(tc.tile_pool(name="psU", bufs=1, space="PSUM"))

        ones_col_f32 = const.tile([128, H], F32, tag="ones_col", name="ones_col")
        nc.vector.memset(ones_col_f32[:], 1.0)
        ones_row_f32 = const.tile([1, 128], F32, tag="ones_row_f", name="ones_row_f")
        nc.vector.memset(ones_row_f32[:], 1.0)
        ones_row = const.tile([1, 128], MMD, tag="ones_row", name="ones_row")
        nc.vector.tensor_copy(ones_row[:], ones_row_f32[:])

        # PE warm-up: dependency-free matmuls so the HAM clock gate opens
        # while the first DMAs stream in.
        warm_w_f = const.tile([128, 128], F32, tag="warm_w_f", name="warm_w_f")
        nc.vector.memset(warm_w_f[:], 0.5)
        warm_w = const.tile([128, 128], MMD, tag="warm_w", name="warm_w")
        nc.vector.tensor_copy(warm_w[:], warm_w_f[:])
        warm_x = const.tile([128, 512], MMD, tag="warm_x", name="warm_x")
        for i in range(4):
            nc.vector.tensor_copy(warm_x[:, i * 128:(i + 1) * 128], warm_w_f[:])
        for r in range(14):
            wp = psS.tile([128, 1024], F32, tag="S", name="S")
            nc.tensor.matmul(wp[:, 0:512], warm_w[:], warm_x[:],
                             start=True, stop=True)
            nc.tensor.matmul(wp[:, 512:1024], warm_w[:], warm_x[:],
                             start=True, stop=True)

        # ---- persistent weights ----
        WqT = wqp.tile([128, CC, C], MMD, tag="wqt", name="wqt")
        WpT_a = wqp.tile([128, H, C], MMD, tag="wpa", name="wpa")
        WpT_b = [wqp.tile([128, C], MMD, tag=f"wpb{g}", name=f"wpb{g}")
                 for g in range(2)]
        bp_row = wqp.tile([1, C], MMD, tag="bp_row", name="bp_row")

        khT_a = [khp.tile([128, N], MMD, tag=f"kha{h}", name=f"kha{h}")
                 for h in range(H)]
        khT_b = [khp.tile([128, N], MMD, tag=f"khb{g}", name=f"khb{g}")
                 for g in range(2)]
        vh = [vhp.tile([128, H, DH + 1], MMD, tag=f"vh{nt}", name=f"vh{nt}")
              for nt in range(KT)]

        def load_wT_grouped(dest, w_dram):
            # dest[p, cc, j] = W.T[cc*128+p, j]
            nc.gpsimd.dma_start(
                dest[:],
                w_dram.rearrange("(cc p) j -> p cc j", p=128))

        def seg_dest(kind, idx, dlo, dhi, a_tiles, b_tiles, col_lo, col_hi):
            t = a_tiles[idx] if kind == "a" else b_tiles[idx]
            return t[dlo:dhi, col_lo:col_hi]

        # ---- phase 1: stage k, then v ------------------------------------
        # One gpsimd cast-DMA queue; ordered WkT, k0..k3, WvT, v0..v3, WqT,
        # qT0, WpT, bias so the first projection starts ~11us in (covered by
        # the warm-up) and nothing later stalls on the queue.
        with tc.tile_pool(name="stg", bufs=1) as stg:
            WkT = stg.tile([128, CC, C], MMD, tag="wkt", name="wkt")
            WvT = stg.tile([128, CC, C], MMD, tag="wvt", name="wvt")
            load_wT_grouped(WkT, wkt_d)

            for ch in range(NCHUNKS):
                n0 = ch * CHUNK
                kTt = stg.tile([128, CC, CHUNK], MMD, tag="xT", name="kTt",
                               bufs=2)
                nc.gpsimd.dma_start(
                    kTt[:],
                    kt_d[:, n0:n0 + CHUNK].rearrange("(cc p) n -> p cc n", p=128))
                if ch == 2:
                    load_wT_grouped(WvT, wvt_d)
                for jc0 in range(0, CC, 2):
                    ps = psS.tile([128, 1024], F32, tag="S", name="S")
                    for cc in range(CC):
                        for i in range(2):
                            nc.tensor.matmul(
                                ps[:, i * 512:(i + 1) * 512],
                                WkT[:, cc, (jc0 + i) * 128:(jc0 + i + 1) * 128],
                                kTt[:, cc, :], start=(cc == 0),
                                stop=(cc == CC - 1))
                    for i in range(2):
                        for (plo, phi, kind, idx, dlo) in _jc_segments(jc0 + i):
                            nc.vector.tensor_copy(
                                seg_dest(kind, idx, dlo, dlo + (phi - plo),
                                         khT_a, khT_b, n0, n0 + CHUNK),
                                ps[plo:phi, i * 512:(i + 1) * 512])

            for ch in range(NCHUNKS):
                n0 = ch * CHUNK
                vTt = stg.tile([128, CC, CHUNK], MMD, tag="xT", name="vTt",
                               bufs=2)
                nc.gpsimd.dma_start(
                    vTt[:],
                    vt_d[:, n0:n0 + CHUNK].rearrange("(cc p) n -> p cc n", p=128))
                if ch == 0:
                    load_wT_grouped(WqT, wqt_d)
                elif ch == 1:
                    # wpt_d is host-packed head-major: rows 0..511 = per-head
                    # dd 0..127 (h-major), rows 512..639 / 640..767 = the
                    # packed b-tiles (dd 128..191 of heads 0,1 / 2,3).
                    nc.gpsimd.dma_start(
                        WpT_a[:],
                        wpt_d[0:512, :].rearrange("(h p) j -> p h j", p=128))
                    for g in range(2):
                        nc.gpsimd.dma_start(
                            WpT_b[g][:],
                            wpt_d[512 + g * 128:512 + (g + 1) * 128, :])
                    nc.gpsimd.dma_start(bp_row[:], bp_d[None, :])
                for ntl in range(4):
                    nt = ch * 4 + ntl
                    ps = psS.tile([128, 1024], F32, tag="S", name="S")
                    for cc in range(CC):
                        for jg in range(2):
                            nc.tensor.matmul(
                                ps[:, jg * 512:jg * 512 + 384],
                                vTt[:, cc, ntl * 128:(ntl + 1) * 128],
                                WvT[:, cc, jg * 384:(jg + 1) * 384],
                                start=(cc == 0), stop=(cc == CC - 1))
                    for jg in range(2):
                        nc.vector.tensor_copy(
                            vh[nt][:, 2 * jg:2 * jg + 2, 0:DH],
                            ps[:, jg * 512:jg * 512 + 384].rearrange(
                                "p (h d) -> p h d", h=2))
                    nc.vector.tensor_copy(
                        vh[nt][:, :, DH:DH + 1],
                        ones_col_f32[:].rearrange("p (h o) -> p h o", h=H))

        # ---- phase 2 pools ----------------------------------------------
        qtp = ctx.enter_context(tc.tile_pool(name="qtp", bufs=1))
        qhp = ctx.enter_context(tc.tile_pool(name="qhp", bufs=1))
        esp = ctx.enter_context(tc.tile_pool(name="esp", bufs=2))
        xop = ctx.enter_context(tc.tile_pool(name="xop", bufs=1))
        scp = ctx.enter_context(tc.tile_pool(name="scp", bufs=1))
        bcp = ctx.enter_context(tc.tile_pool(name="bcp", bufs=1))
        yp = ctx.enter_context(tc.tile_pool(name="yp", bufs=2))

        def q_load(qc):
            n0 = qc * CHUNK
            qTt = qtp.tile([128, CC, CHUNK], MMD, tag="qTt", name="qTt")
            nc.gpsimd.dma_start(
                qTt[:],
                qt_d[:, n0:n0 + CHUNK].rearrange("(cc p) n -> p cc n", p=128))
            return qTt


        xT_a = [xop.tile([128, CHUNK], MMD, tag=f"xta{h}", name=f"xta{h}")
                for h in range(H)]
        xT_b = [xop.tile([128, CHUNK], MMD, tag=f"xtb{g}", name=f"xtb{g}")
                for g in range(2)]

        def q_proj(qTt, interleave=None):
            """Project one q chunk. `interleave(j)` is called between
            jc-groups to slot independent PE/DVE work into the stream."""
            qhT_a = [qhp.tile([128, CHUNK], MMD, tag=f"qha{h}", name=f"qha{h}")
                     for h in range(H)]
            qhT_b = [qhp.tile([128, CHUNK], MMD, tag=f"qhb{g}", name=f"qhb{g}")
                     for g in range(2)]
            for j, jc0 in enumerate(range(0, CC, 2)):
                ps = psS.tile([128, 1024], F32, tag="S", name="S")
                for cc in range(CC):
                    for i in range(2):
                        nc.tensor.matmul(
                            ps[:, i * 512:(i + 1) * 512],
                            WqT[:, cc, (jc0 + i) * 128:(jc0 + i + 1) * 128],
                            qTt[:, cc, :], start=(cc == 0), stop=(cc == CC - 1))
                for i in range(2):
                    for (plo, phi, kind, idx, dlo) in _jc_segments(jc0 + i):
                        nc.vector.tensor_copy(
                            seg_dest(kind, idx, dlo, dlo + (phi - plo),
                                     qhT_a, qhT_b, 0, CHUNK),
                            ps[plo:phi, i * 512:(i + 1) * 512])
                if interleave is not None:
                    interleave(j)
            return qhT_a, qhT_b

        def pair_attention(g, qhT_a, qhT_b, qc_end=False, interleave=None):
            """One head pair (heads 2g, 2g+1) over all 16 k-tiles.
            Scores run TWO k-tiles ahead so the exp ACTIVATE has a full
            iteration of slack - av(kt) never waits on ScalarE even though
            the Tile scheduler orders av before scores in the PE queue.
            Returns the two heads' f32r reciprocal rows."""
            h0, h1 = 2 * g, 2 * g + 1
            uA0 = psU.tile([128, 512], F32, tag="uA0", name="uA0")
            uB0 = psU.tile([65, 512], F32, tag="uB0", name="uB0")
            uA1 = psU.tile([128, 512], F32, tag="uA1", name="uA1")
            uB1 = psU.tile([65, 512], F32, tag="uB1", name="uB1")
            es_t = [None] * KT

            def scores(kt):
                S = psS.tile([128, 1024], F32, tag="S", name="S")
                nc.tensor.matmul(
                    S[:, 0:512], khT_a[h0][:, kt * 128:(kt + 1) * 128],
                    qhT_a[h0][:], start=True, stop=False)
                nc.tensor.matmul(
                    S[:, 512:1024], khT_a[h1][:, kt * 128:(kt + 1) * 128],
                    qhT_a[h1][:], start=True, stop=False)
                # the two K=64 remainders sit at array row groups 0-1 / 2-3
                # (base partitions 0 / 64) and are adjacent in the stream, so
                # they execute concurrently.
                nc.tensor.matmul(
                    S[:, 0:512], khT_b[g][0:64, kt * 128:(kt + 1) * 128],
                    qhT_b[g][0:64, :], start=False, stop=True)
                nc.tensor.matmul(
                    S[:, 512:1024], khT_b[g][64:128, kt * 128:(kt + 1) * 128],
                    qhT_b[g][64:128, :], start=False, stop=True)
                es = esp.tile([128, 1024], MMD, tag="es", name="es")
                nc.scalar.activation(es[:], S[:], AF.Exp, scale=SCALE)
                es_t[kt] = es

            def av(kt):
                es = es_t[kt]
                st, sp = kt == 0, kt == KT - 1
                nc.tensor.matmul(uA0[:], vh[kt][:, h0, 0:128], es[:, 0:512],
                                 start=st, stop=sp)
                nc.tensor.matmul(uB0[:], vh[kt][:, h0, 128:DH + 1],
                                 es[:, 0:512], start=st, stop=sp)
                nc.tensor.matmul(uA1[:], vh[kt][:, h1, 0:128],
                                 es[:, 512:1024], start=st, stop=sp)
                nc.tensor.matmul(uB1[:], vh[kt][:, h1, 128:DH + 1],
                                 es[:, 512:1024], start=st, stop=sp)
                es_t[kt] = None

            scores(0)
            scores(1)
            for kt in range(KT):
                if kt + 2 < KT:
                    scores(kt + 2)
                av(kt)
                if interleave is not None and kt in interleave:
                    interleave[kt]()

            # boundary: evacuate U unnormalized (frees the u banks), bounce
            # the rowsum rows to SBUF, reciprocal.  Mid-qc the next pair
            # reuses the u banks within ~2 iterations, so the big copies go
            # first (uA0 frees first); at the qc end the recip chain gates
            # the bc matmuls, so the rowsum rows go first.
            def rchain(h, uB):
                row = scp.tile([1, 512], F32, tag="row", name="row")
                nc.vector.tensor_copy(row[:], uB[64:65, :])
                rt = scp.tile([1, 512], F32, tag="rt", name="rt")
                nc.vector.reciprocal_approx_fast(rt[:], row[:])
                rtr = scp.tile([1, 512], MMD, tag=f"rtr{h}", name=f"rtr{h}")
                nc.vector.tensor_copy(rtr[:], rt[:])
                return rtr

            rtrs = []
            if qc_end:
                for (h, uA, uB, blo) in ((h0, uA0, uB0, 0), (h1, uA1, uB1, 64)):
                    rtrs.append(rchain(h, uB))
                for (h, uA, uB, blo) in ((h0, uA0, uB0, 0), (h1, uA1, uB1, 64)):
                    nc.vector.tensor_copy(xT_a[h][:], uA[:])
                    nc.vector.tensor_copy(xT_b[g][blo:blo + 64, :], uB[0:64, :])
            else:
                for (h, uA, uB, blo) in ((h0, uA0, uB0, 0), (h1, uA1, uB1, 64)):
                    nc.vector.tensor_copy(xT_a[h][:], uA[:])
                    nc.vector.tensor_copy(xT_b[g][blo:blo + 64, :], uB[0:64, :])
                    rtrs.append(rchain(h, uB))
            return rtrs

        def bc_normalize(g, rtrs):
            """Rank-1 broadcast of 1/r for heads 2g,2g+1 + in-place
            normalization of their xT tiles.  The broadcasts use the psU
            banks (free at the qc boundary) so their DVE evac chain never
            entangles the S pool's WAR rotation - with bc in the S pool the
            boundary q_proj matmuls inherited a wait on the whole boundary
            DVE chain (~5us PE stall, enough to re-throttle the clock)."""
            h0, h1 = 2 * g, 2 * g + 1
            psa = psU.tile([128, 512], F32, tag="uA0", name="uA0")
            psb = psU.tile([128, 512], F32, tag="uA1", name="uA1")
            nc.tensor.matmul(psa[:], ones_row[:], rtrs[0][:],
                             start=True, stop=True)
            nc.tensor.matmul(psb[:], ones_row[:], rtrs[1][:],
                             start=True, stop=True)
            bc = bcp.tile([128, 1024], MMD, tag="bc", name="bc")
            nc.vector.tensor_copy(bc[:, 0:512], psa[:])
            nc.vector.tensor_copy(bc[:, 512:1024], psb[:])
            nc.vector.tensor_mul(xT_a[h0][:], xT_a[h0][:], bc[:, 0:512])
            nc.vector.tensor_mul(xT_b[g][0:64, :], xT_b[g][0:64, :],
                                 bc[0:64, 0:512])
            nc.vector.tensor_mul(xT_a[h1][:], xT_a[h1][:], bc[:, 512:1024])
            nc.vector.tensor_mul(xT_b[g][64:128, :], xT_b[g][64:128, :],
                                 bc[64:128, 512:1024])

        def final_proj(qc):
            n0 = qc * CHUNK
            for ntl in range(4):
                ps = psS.tile([128, 1024], F32, tag="S", name="S")
                # bias as a K=1 ones-row matmul opening both accumulation
                # groups (j 0:512 in bank 0, j 512:768 in bank 1)
                nc.tensor.matmul(ps[:, 0:512], ones_row[:], bp_row[:, 0:512],
                                 start=True, stop=False)
                nc.tensor.matmul(ps[:, 512:768], ones_row[:], bp_row[:, 512:C],
                                 start=True, stop=False)
                for h in range(H):
                    blo = (h % 2) * 64
                    last = h == H - 1
                    nc.tensor.matmul(
                        ps[:, 0:512],
                        xT_a[h][:, ntl * 128:(ntl + 1) * 128],
                        WpT_a[:, h, 0:512], start=False, stop=False)
                    nc.tensor.matmul(
                        ps[:, 512:768],
                        xT_a[h][:, ntl * 128:(ntl + 1) * 128],
                        WpT_a[:, h, 512:C], start=False, stop=False)
                    nc.tensor.matmul(
                        ps[:, 0:512],
                        xT_b[h // 2][blo:blo + 64, ntl * 128:(ntl + 1) * 128],
                        WpT_b[h // 2][blo:blo + 64, 0:512],
                        start=False, stop=last)
                    nc.tensor.matmul(
                        ps[:, 512:768],
                        xT_b[h // 2][blo:blo + 64, ntl * 128:(ntl + 1) * 128],
                        WpT_b[h // 2][blo:blo + 64, 512:C],
                        start=False, stop=last)
                ysb = yp.tile([128, C], F32, tag="y", name="y")
                nc.scalar.copy(ysb[:], ps[:, 0:C])
                nc.sync.dma_start(
                    y_d[n0 + ntl * 128:n0 + (ntl + 1) * 128, :], ysb[:])

        # ---- phase 2: per q-chunk attention + projections ------------------
        qh = q_proj(q_load(0))
        for qc in range(NCHUNKS):
            last_qc = qc + 1 == NCHUNKS
            if not last_qc:
                qt_next = q_load(qc + 1)
            rtrs0 = pair_attention(0, *qh, qc_end=False)
            if last_qc:
                # no q_proj to cover the boundary: pre-normalize pair 0
                # mid-pair-1 (one S-rotation hiccup, but final_proj's h0/h1
                # matmuls become ready work at the boundary)
                rtrs1 = pair_attention(
                    1, *qh, qc_end=True,
                    interleave={4: (lambda _r0=rtrs0: bc_normalize(0, _r0))})
                bc_normalize(1, rtrs1)
            else:
                rtrs1 = pair_attention(1, *qh, qc_end=True)
                # qc boundary: the next chunk's q projection leads the PE
                # queue (independent work); the bc chains slot in after
                # groups 0/1, by which time the DVE recip chains are done.
                def inter(j, _r0=rtrs0, _r1=rtrs1):
                    if j == 0:
                        bc_normalize(0, _r0)
                    elif j == 1:
                        bc_normalize(1, _r1)
                qh = q_proj(qt_next, interleave=inter)
            final_proj(qc)

    nc.compile()
    return nc


def _get_built():
    global _BUILT
    if _BUILT is None:
        _BUILT = _build()
    return _BUILT


def run(inputs, trace=False, **kw):
    """Run on all 8 cores; returns (y [B,N,C] float32, BassKernelResults)."""
    from concourse.bass_utils import run_bass_kernel_spmd

    nc = _get_built()
    f32 = np.float32
    wpt = np.asarray(inputs["Wp"], f32).T  # [c', j]
    wpt_packed = np.concatenate(
        [wpt[h * DH:h * DH + 128] for h in range(H)]
        + [wpt[h * DH + 128:(h + 1) * DH] for h in range(H)])
    shared = {
        "WqT": np.ascontiguousarray(np.asarray(inputs["Wq"], f32).T),
        "WkT": np.ascontiguousarray(np.asarray(inputs["Wk"], f32).T),
        "WvT": np.ascontiguousarray(np.asarray(inputs["Wv"], f32).T),
        "WpT": np.ascontiguousarray(wpt_packed),
        "bp": np.ascontiguousarray(np.asarray(inputs["bp"], f32)),
    }
    q = np.asarray(inputs["q"], f32)
    k = np.asarray(inputs["k"], f32)
    v = np.asarray(inputs["v"], f32)
    in_maps = []
    for b in range(B):
        m = dict(shared)
        m["qT"] = np.ascontiguousarray(q[b].T)
        m["kT"] = np.ascontiguousarray(k[b].T)
        m["vT"] = np.ascontiguousarray(v[b].T)
        in_maps.append(m)
    res = run_bass_kernel_spmd(nc, in_maps, list(range(B)), trace=trace, **kw)
    y = np.stack([res.results[b]["y"] for b in range(B)]).astype(np.float32)
    return y, res


def kernel(q, k, v, Wq, Wk, Wv, Wp, bp):
    y, _ = run({"q": q, "k": k, "v": v, "Wq": Wq, "Wk": Wk, "Wv": Wv,
                "Wp": Wp, "bp": bp})
    return y


# revision 21
# speedup vs baseline: 1.0669x; 1.0491x over previous
"""Trainium2 Bass kernel for nn_Attention_44994077393310.

Multi-head attention (B=8, N=2048, C=768, H=4, Dh=192) with input projections,
softmax attention, and output projection with bias.

Sharding: pure data-parallel over the batch dim - each of the 8 NeuronCores
computes one batch element end-to-end (weights replicated). No collectives.

v2 design notes (vs the 643us baseline):
  - The baseline ran ~45% of the kernel at the PE's cold clock (HAM K=4/8,
    1.2 GHz): the scores->exp->AV inner loop had ~35% PE micro-idle per step,
    which kept re-throttling the clock gate. This version restructures the
    attention loop around HEAD PAIRS so the PE stream is dense:
      * the two K=64 score remainder matmuls of a head pair sit at array row
        groups 0-1 / 2-3 (base partitions 0/64) and are issued back-to-back,
        so they run CONCURRENTLY (row-tiled packing);
      * one 2-bank PSUM tile [128,1024] holds the pair's scores, and a single
        ACTIVATE exps both heads (1147ns vs 2x720ns), keeping ScalarE off the
        critical path;
      * softmax denominators: 1/rowsum via reciprocal_approx_fast (DVE custom
        op, ~0.7us vs 3.35us for the iterative divide), with the PSUM->SBUF
        bounce the op requires;
      * U is evacuated UNNORMALIZED with plain DVE copies right at the pair
        boundary (frees the 4 u PSUM banks for the next pair within ~2us);
        the 1/r normalization happens later as in-place DVE multiplies
        against a PE rank-1 broadcast, entirely off the critical path.
  - q is projected per chunk at the PREVIOUS chunk boundary (PE-dense work
    that covers the finalize chains); q/k/v/final projections and the rank-1
    broadcasts all share the score PSUM pool, so PSUM is exactly 8 banks.
  - the output projection folds the bias in as a K=1 ones-row matmul and
    evacuates y on ScalarE (idle at boundaries), keeping DVE under budget.
"""

import numpy as np

B = 8
N = 2048
C = 768
H = 4
DH = 192
SCALE = DH ** -0.5

NCHUNKS = 4                # chunks of 512 over the sequence
CHUNK = N // NCHUNKS       # 512
CC = C // 128              # 6 channel chunks
KT = N // 128              # 16 k-tiles

_BUILT = None


def _dest_of(cp):
    h, dd = divmod(cp, DH)
    if dd < 128:
        return ("a", h, dd)
    return ("b", h // 2, (h % 2) * 64 + (dd - 128))


def _jc_segments(jc):
    """Merged PSUM->head-major copy segments for projection j-chunk jc."""
    segs = []
    for p0 in range(0, 128, 64):
        kind, idx, dlo = _dest_of(128 * jc + p0)
        if segs and segs[-1][2] == kind and segs[-1][3] == idx and \
                segs[-1][4] + (segs[-1][1] - segs[-1][0]) == dlo:
            segs[-1] = (segs[-1][0], p0 + 64, kind, idx, segs[-1][4])
        else:
            segs.append((p0, p0 + 64, kind, idx, dlo))
    return segs


def _build():
    from contextlib import ExitStack

    import concourse.mybir as mybir
    import concourse.tile as tile
    from concourse import bacc

    F32 = mybir.dt.float32
    MMD = mybir.dt.float32r
    AF = mybir.ActivationFunctionType

    nc = bacc.Bacc("TRN2", target_bir_lowering=False, debug=False)
    qt_d = nc.dram_tensor("qT", [C, N], F32, kind="ExternalInput").ap()
    kt_d = nc.dram_tensor("kT", [C, N], F32, kind="ExternalInput").ap()
    vt_d = nc.dram_tensor("vT", [C, N], F32, kind="ExternalInput").ap()
    wqt_d = nc.dram_tensor("WqT", [C, C], F32, kind="ExternalInput").ap()
    wkt_d = nc.dram_tensor("WkT", [C, C], F32, kind="ExternalInput").ap()
    wvt_d = nc.dram_tensor("WvT", [C, C], F32, kind="ExternalInput").ap()
    wpt_d = nc.dram_tensor("WpT", [C, C], F32, kind="ExternalInput").ap()
    bp_d = nc.dram_tensor("bp", [C], F32, kind="ExternalInput").ap()
    y_d = nc.dram_tensor("y", [N, C], F32, kind="ExternalOutput").ap()

    with tile.TileContext(nc) as tc, ExitStack() as ctx:
        const = ctx.enter_context(tc.tile_pool(name="const", bufs=1))
        wqp = ctx.enter_context(tc.tile_pool(name="wqp", bufs=1))
        khp = ctx.enter_context(tc.tile_pool(name="khp", bufs=1))
        vhp = ctx.enter_context(tc.tile_pool(name="vhp", bufs=1))
        psS = ctx.enter_context(tc.tile_pool(name="psS", bufs=2, space="PSUM"))
        psU = ctx.enter_context(tc.tile_pool(name="psU", bufs=1, space="PSUM"))

        ones_col_f32 = const.tile([128, H], F32, tag="ones_col", name="ones_col")
        nc.vector.memset(ones_col_f32[:], 1.0)
        ones_row = const.tile([1, 128], MMD, tag="ones_row", name="ones_row")
        nc.vector.memset(ones_row[:].bitcast(F32), 1.0)

        # PE warm-up: dependency-free matmuls so the HAM clock gate opens
        # while the first DMAs stream in (memset constants are exactly
        # representable, so the fp32r-rounding verifier is satisfied).
        warm_w = const.tile([128, 128], MMD, tag="warm_w", name="warm_w")
        nc.vector.memset(warm_w[:].bitcast(F32), 0.5)
        warm_x = const.tile([128, 256], MMD, tag="warm_x", name="warm_x")
        nc.vector.memset(warm_x[:].bitcast(F32), 0.5)
        for r in range(48):
            wp = psS.tile([128, 1024], F32, tag="S", name="S")
            nc.tensor.matmul(wp[:, 0:256], warm_w[:], warm_x[:],
                             start=True, stop=True)
            nc.tensor.matmul(wp[:, 512:768], warm_w[:], warm_x[:],
                             start=True, stop=True)

        # ---- persistent weights ----
        WqT = wqp.tile([128, CC, C], MMD, tag="wqt", name="wqt")
        WpT_a = wqp.tile([128, H, C], MMD, tag="wpa", name="wpa")
        WpT_b = [wqp.tile([128, C], MMD, tag=f"wpb{g}", name=f"wpb{g}")
                 for g in range(2)]
        bp_row = wqp.tile([1, C], MMD, tag="bp_row", name="bp_row")

        khT_a = [khp.tile([128, N], MMD, tag=f"kha{h}", name=f"kha{h}")
                 for h in range(H)]
        khT_b = [khp.tile([128, N], MMD, tag=f"khb{g}", name=f"khb{g}")
                 for g in range(2)]
        vh = [vhp.tile([128, H, DH + 1], MMD, tag=f"vh{nt}", name=f"vh{nt}")
              for nt in range(KT)]
        for nt in range(KT):
            nc.vector.tensor_copy(
                vh[nt][:, :, DH:DH + 1],
                ones_col_f32[:].rearrange("p (h o) -> p h o", h=H))

        def load_wT_grouped(dest, w_dram):
            # dest[p, cc, j] = W.T[cc*128+p, j]
            nc.gpsimd.dma_start(
                dest[:],
                w_dram.rearrange("(cc p) j -> p cc j", p=128))

        def seg_dest(kind, idx, dlo, dhi, a_tiles, b_tiles, col_lo, col_hi):
            t = a_tiles[idx] if kind == "a" else b_tiles[idx]
            return t[dlo:dhi, col_lo:col_hi]

        # ---- phase 1: stage k, then v ------------------------------------
        # One gpsimd cast-DMA queue; ordered WkT, k0..k3, WvT, v0..v3, WqT,
        # qT0, WpT, bias so the first projection starts ~11us in (covered by
        # the warm-up) and nothing later stalls on the queue.
        with tc.tile_pool(name="stg", bufs=1) as stg:
            WkT = stg.tile([128, CC, C], MMD, tag="wkt", name="wkt")
            WvT = stg.tile([128, CC, C], MMD, tag="wvt", name="wvt")
            load_wT_grouped(WkT, wkt_d)

            for ch in range(NCHUNKS):
                n0 = ch * CHUNK
                kTt = stg.tile([128, CC, CHUNK], MMD, tag="xT", name="kTt",
                               bufs=2)
                nc.gpsimd.dma_start(
                    kTt[:],
                    kt_d[:, n0:n0 + CHUNK].rearrange("(cc p) n -> p cc n", p=128))
                if ch == 2:
                    load_wT_grouped(WvT, wvt_d)
                for jc0 in range(0, CC, 2):
                    ps = psS.tile([128, 1024], F32, tag="S", name="S")
                    for cc in range(CC):
                        for i in range(2):
                            nc.tensor.matmul(
                                ps[:, i * 512:(i + 1) * 512],
                                WkT[:, cc, (jc0 + i) * 128:(jc0 + i + 1) * 128],
                                kTt[:, cc, :], start=(cc == 0),
                                stop=(cc == CC - 1))
                    for i in range(2):
                        for (plo, phi, kind, idx, dlo) in _jc_segments(jc0 + i):
                            nc.vector.tensor_copy(
                                seg_dest(kind, idx, dlo, dlo + (phi - plo),
                                         khT_a, khT_b, n0, n0 + CHUNK),
                                ps[plo:phi, i * 512:(i + 1) * 512])

            for ch in range(NCHUNKS):
                n0 = ch * CHUNK
                vTt = stg.tile([128, CC, CHUNK], MMD, tag="xT", name="vTt",
                               bufs=2)
                nc.gpsimd.dma_start(
                    vTt[:],
                    vt_d[:, n0:n0 + CHUNK].rearrange("(cc p) n -> p cc n", p=128))
                if ch == 0:
                    load_wT_grouped(WqT, wqt_d)
                elif ch == 1:
                    # wpt_d is host-packed head-major: rows 0..511 = per-head
                    # dd 0..127 (h-major), rows 512..639 / 640..767 = the
                    # packed b-tiles (dd 128..191 of heads 0,1 / 2,3).
                    nc.gpsimd.dma_start(
                        WpT_a[:],
                        wpt_d[0:512, :].rearrange("(h p) j -> p h j", p=128))
                    for g in range(2):
                        nc.gpsimd.dma_start(
                            WpT_b[g][:],
                            wpt_d[512 + g * 128:512 + (g + 1) * 128, :])
                    nc.gpsimd.dma_start(bp_row[:], bp_d[None, :])
                for ntl in range(4):
                    nt = ch * 4 + ntl
                    ps = psS.tile([128, 1024], F32, tag="S", name="S")
                    for cc in range(CC):
                        for jg in range(2):
                            nc.tensor.matmul(
                                ps[:, jg * 512:jg * 512 + 384],
                                vTt[:, cc, ntl * 128:(ntl + 1) * 128],
                                WvT[:, cc, jg * 384:(jg + 1) * 384],
                                start=(cc == 0), stop=(cc == CC - 1))
                    for jg in range(2):
                        # ScalarE evac: keeps the DVE queue short so the
                        # phase-1 tail does not gate q_proj(0)'s S tiles
                        nc.scalar.copy(
                            vh[nt][:, 2 * jg:2 * jg + 2, 0:DH],
                            ps[:, jg * 512:jg * 512 + 384].rearrange(
                                "p (h d) -> p h d", h=2))

        # ---- phase 2 pools ----------------------------------------------
        qtp = ctx.enter_context(tc.tile_pool(name="qtp", bufs=1))
        qhp = ctx.enter_context(tc.tile_pool(name="qhp", bufs=1))
        esp = ctx.enter_context(tc.tile_pool(name="esp", bufs=2))
        xop = ctx.enter_context(tc.tile_pool(name="xop", bufs=1))
        scp = ctx.enter_context(tc.tile_pool(name="scp", bufs=1))
        bcp = ctx.enter_context(tc.tile_pool(name="bcp", bufs=1))
        yp = ctx.enter_context(tc.tile_pool(name="yp", bufs=2))

        def q_load(qc):
            n0 = qc * CHUNK
            qTt = qtp.tile([128, CC, CHUNK], MMD, tag="qTt", name="qTt")
            nc.gpsimd.dma_start(
                qTt[:],
                qt_d[:, n0:n0 + CHUNK].rearrange("(cc p) n -> p cc n", p=128))
            return qTt


        xT_a = [xop.tile([128, CHUNK], MMD, tag=f"xta{h}", name=f"xta{h}")
                for h in range(H)]
        xT_b = [xop.tile([128, CHUNK], MMD, tag=f"xtb{g}", name=f"xtb{g}")
                for g in range(2)]

        qhT_bp = [qhp.tile([128, CHUNK], MMD, tag=f"qhb{h}", name=f"qhb{h}")
                  for h in range(H)]
        for h in range(H):
            blo = (h % 2) * 64
            nc.vector.memset(qhT_bp[h][(64 - blo):(128 - blo), :].bitcast(F32),
                             0.0)

        def q_proj(qTt, interleave=None):
            """Project one q chunk. `interleave(j)` is called between
            jc-groups to slot independent PE/DVE work into the stream."""
            qhT_a = [qhp.tile([128, CHUNK], MMD, tag=f"qha{h}", name=f"qha{h}")
                     for h in range(H)]
            # per-head b tiles, zero-padded to K=128 (persistent tiles:
            # head 2g's data sits in rows 0-63 and 2g+1's in rows 64-127,
            # matching khT_b's packing; the other half stays zero forever).
            # The score b-matmuls are then full-K with the SAME stationary
            # operand for both heads of a pair - a uniform LDW-pipelined
            # stream instead of row-tiled pairs whose LDWEIGHTS bubbles kept
            # re-throttling the clock gate.
            qhT_b = qhT_bp
            for j, jc0 in enumerate(range(0, CC, 2)):
                ps = psS.tile([128, 1024], F32, tag="S", name="S")
                for cc in range(CC):
                    for i in range(2):
                        nc.tensor.matmul(
                            ps[:, i * 512:(i + 1) * 512],
                            WqT[:, cc, (jc0 + i) * 128:(jc0 + i + 1) * 128],
                            qTt[:, cc, :], start=(cc == 0), stop=(cc == CC - 1))
                for i in range(2):
                    for (plo, phi, kind, idx, dlo) in _jc_segments(jc0 + i):
                        if kind == "a":
                            dest = qhT_a[idx][dlo:dlo + (phi - plo), 0:CHUNK]
                        else:
                            h = 2 * idx + (1 if dlo >= 64 else 0)
                            dest = qhT_b[h][dlo:dlo + (phi - plo), 0:CHUNK]
                        nc.vector.tensor_copy(
                            dest, ps[plo:phi, i * 512:(i + 1) * 512])
                if interleave is not None:
                    interleave(j)
            return qhT_a, qhT_b

        def pair_attention(g, qhT_a, qhT_b, qc_end=False, interleave=None):
            """One head pair (heads 2g, 2g+1) over all 16 k-tiles.
            Scores run TWO k-tiles ahead so the exp ACTIVATE has a full
            iteration of slack - av(kt) never waits on ScalarE even though
            the Tile scheduler orders av before scores in the PE queue.
            Returns the two heads' f32r reciprocal rows."""
            h0, h1 = 2 * g, 2 * g + 1
            uA0 = psU.tile([128, 512], F32, tag="uA0", name="uA0")
            uB0 = psU.tile([65, 512], F32, tag="uB0", name="uB0")
            uA1 = psU.tile([128, 512], F32, tag="uA1", name="uA1")
            uB1 = psU.tile([65, 512], F32, tag="uB1", name="uB1")
            es_t = [None] * KT

            def scores(kt):
                S = psS.tile([128, 1024], F32, tag="S", name="S")
                nc.tensor.matmul(
                    S[:, 0:512], khT_a[h0][:, kt * 128:(kt + 1) * 128],
                    qhT_a[h0][:], start=True, stop=False)
                nc.tensor.matmul(
                    S[:, 512:1024], khT_a[h1][:, kt * 128:(kt + 1) * 128],
                    qhT_a[h1][:], start=True, stop=False)
                # b remainders as full-K matmuls against the zero-padded
                # per-head q tiles; both share the packed khT_b stationary
                nc.tensor.matmul(
                    S[:, 0:512], khT_b[g][:, kt * 128:(kt + 1) * 128],
                    qhT_b[h0][:], start=False, stop=True)
                nc.tensor.matmul(
                    S[:, 512:1024], khT_b[g][:, kt * 128:(kt + 1) * 128],
                    qhT_b[h1][:], start=False, stop=True)
                es = esp.tile([128, 1024], MMD, tag="es", name="es")
                nc.scalar.activation(es[:], S[:], AF.Exp, scale=SCALE)
                es_t[kt] = es

            def av(kt):
                es = es_t[kt]
                st, sp = kt == 0, kt == KT - 1
                nc.tensor.matmul(uA0[:], vh[kt][:, h0, 0:128], es[:, 0:512],
                                 start=st, stop=sp)
                nc.tensor.matmul(uB0[:], vh[kt][:, h0, 128:DH + 1],
                                 es[:, 0:512], start=st, stop=sp)
                nc.tensor.matmul(uA1[:], vh[kt][:, h1, 0:128],
                                 es[:, 512:1024], start=st, stop=sp)
                nc.tensor.matmul(uB1[:], vh[kt][:, h1, 128:DH + 1],
                                 es[:, 512:1024], start=st, stop=sp)
                es_t[kt] = None

            scores(0)
            scores(1)
            for kt in range(KT):
                if kt + 2 < KT:
                    scores(kt + 2)
                av(kt)
                if interleave is not None and kt in interleave:
                    interleave[kt]()

            # boundary: evacuate U unnormalized (frees the u banks), bounce
            # the rowsum rows to SBUF, reciprocal.  Mid-qc the next pair
            # reuses the u banks within ~2 iterations, so the big copies go
            # first (uA0 frees first); at the qc end the recip chain gates
            # the bc matmuls, so the rowsum rows go first.
            def rchain(h, uB):
                row = scp.tile([1, 512], F32, tag="row", name="row")
                nc.vector.tensor_copy(row[:], uB[64:65, :])
                rt = scp.tile([1, 512], F32, tag="rt", name="rt")
                nc.vector.reciprocal_approx_fast(rt[:], row[:])
                rtr = scp.tile([1, 512], MMD, tag=f"rtr{h}", name=f"rtr{h}")
                nc.vector.tensor_copy(rtr[:], rt[:])
                return rtr

            rtrs = []
            if qc_end:
                for (h, uA, uB, blo) in ((h0, uA0, uB0, 0), (h1, uA1, uB1, 64)):
                    rtrs.append(rchain(h, uB))
                for (h, uA, uB, blo) in ((h0, uA0, uB0, 0), (h1, uA1, uB1, 64)):
                    nc.vector.tensor_copy(xT_a[h][:], uA[:])
                    nc.vector.tensor_copy(xT_b[g][blo:blo + 64, :], uB[0:64, :])
            else:
                for (h, uA, uB, blo) in ((h0, uA0, uB0, 0), (h1, uA1, uB1, 64)):
                    nc.vector.tensor_copy(xT_a[h][:], uA[:])
                    nc.vector.tensor_copy(xT_b[g][blo:blo + 64, :], uB[0:64, :])
                    rtrs.append(rchain(h, uB))
            return rtrs

        def bc_normalize(g, rtrs):
            """Rank-1 broadcast of 1/r for heads 2g,2g+1 + in-place
            normalization of their xT tiles.  The broadcasts use the psU
            banks (free at the qc boundary) so their DVE evac chain never
            entangles the S pool's WAR rotation - with bc in the S pool the
            boundary q_proj matmuls inherited a wait on the whole boundary
            DVE chain (~5us PE stall, enough to re-throttle the clock)."""
            h0, h1 = 2 * g, 2 * g + 1
            psa = psU.tile([128, 512], F32, tag="uA0", name="uA0")
            psb = psU.tile([128, 512], F32, tag="uA1", name="uA1")
            nc.tensor.matmul(psa[:], ones_row[:], rtrs[0][:],
                             start=True, stop=True)
            nc.tensor.matmul(psb[:], ones_row[:], rtrs[1][:],
                             start=True, stop=True)
            bc = bcp.tile([128, 1024], MMD, tag="bc", name="bc")
            nc.vector.tensor_copy(bc[:, 0:512], psa[:])
            nc.vector.tensor_copy(bc[:, 512:1024], psb[:])
            nc.vector.tensor_mul(xT_a[h0][:], xT_a[h0][:], bc[:, 0:512])
            nc.vector.tensor_mul(xT_b[g][0:64, :], xT_b[g][0:64, :],
                                 bc[0:64, 0:512])
            nc.vector.tensor_mul(xT_a[h1][:], xT_a[h1][:], bc[:, 512:1024])
            nc.vector.tensor_mul(xT_b[g][64:128, :], xT_b[g][64:128, :],
                                 bc[64:128, 512:1024])

        def final_proj(qc):
            n0 = qc * CHUNK
            for ntl in range(4):
                ps = psS.tile([128, 1024], F32, tag="S", name="S")
                # bias as a K=1 ones-row matmul opening both accumulation
                # groups (j 0:512 in bank 0, j 512:768 in bank 1)
                nc.tensor.matmul(ps[:, 0:512], ones_row[:], bp_row[:, 0:512],
                                 start=True, stop=False)
                nc.tensor.matmul(ps[:, 512:768], ones_row[:], bp_row[:, 512:C],
                                 start=True, stop=False)
                for h in range(H):
                    blo = (h % 2) * 64
                    last = h == H - 1
                    nc.tensor.matmul(
                        ps[:, 0:512],
                        xT_a[h][:, ntl * 128:(ntl + 1) * 128],
                        WpT_a[:, h, 0:512], start=False, stop=False)
                    nc.tensor.matmul(
                        ps[:, 512:768],
                        xT_a[h][:, ntl * 128:(ntl + 1) * 128],
                        WpT_a[:, h, 512:C], start=False, stop=False)
                    nc.tensor.matmul(
                        ps[:, 0:512],
                        xT_b[h // 2][blo:blo + 64, ntl * 128:(ntl + 1) * 128],
                        WpT_b[h // 2][blo:blo + 64, 0:512],
                        start=False, stop=last)
                    nc.tensor.matmul(
                        ps[:, 512:768],
                        xT_b[h // 2][blo:blo + 64, ntl * 128:(ntl + 1) * 128],
                        WpT_b[h // 2][blo:blo + 64, 512:C],
                        start=False, stop=last)
                ysb = yp.tile([128, C], F32, tag="y", name="y")
                nc.scalar.copy(ysb[:], ps[:, 0:C])
                nc.sync.dma_start(
                    y_d[n0 + ntl * 128:n0 + (ntl + 1) * 128, :], ysb[:])

        # ---- phase 2: per q-chunk attention + projections ------------------
        qh = q_proj(q_load(0))
        for qc in range(NCHUNKS):
            last_qc = qc + 1 == NCHUNKS
            if not last_qc:
                qt_next = q_load(qc + 1)
            rtrs0 = pair_attention(0, *qh, qc_end=False)
            if last_qc:
                # bc uses the psU banks now, so it must NOT be interleaved
                # into pair 1's loop (uA0/uA1 are its live accumulators)
                rtrs1 = pair_attention(1, *qh, qc_end=True)
                bc_normalize(0, rtrs0)
                bc_normalize(1, rtrs1)
            else:
                rtrs1 = pair_attention(1, *qh, qc_end=True)
                # qc boundary: the next chunk's q projection leads the PE
                # queue (independent work); the bc chains slot in after
                # groups 0/1, by which time the DVE recip chains are done.
                def inter(j, _r0=rtrs0, _r1=rtrs1):
                    if j == 0:
                        bc_normalize(0, _r0)
                    elif j == 1:
                        bc_normalize(1, _r1)
                qh = q_proj(qt_next, interleave=inter)
            final_proj(qc)

    nc.compile()
    return nc


def _get_built():
    global _BUILT
    if _BUILT is None:
        _BUILT = _build()
    return _BUILT


def run(inputs, trace=False, **kw):
    """Run on all 8 cores; returns (y [B,N,C] float32, BassKernelResults)."""
    from concourse.bass_utils import run_bass_kernel_spmd

    nc = _get_built()
    f32 = np.float32
    wpt = np.asarray(inputs["Wp"], f32).T  # [c', j]
    wpt_packed = np.concatenate(
        [wpt[h * DH:h * DH + 128] for h in range(H)]
        + [wpt[h * DH + 128:(h + 1) * DH] for h in range(H)])
    shared = {
        "WqT": np.ascontiguousarray(np.asarray(inputs["Wq"], f32).T),
        "WkT": np.ascontiguousarray(np.asarray(inputs["Wk"], f32).T),
        "WvT": np.ascontiguousarray(np.asarray(inputs["Wv"], f32).T),
        "WpT": np.ascontiguousarray(wpt_packed),
        "bp": np.ascontiguousarray(np.asarray(inputs["bp"], f32)),
    }
    q = np.asarray(inputs["q"], f32)
    k = np.asarray(inputs["k"], f32)
    v = np.asarray(inputs["v"], f32)
    in_maps = []
    for b in range(B):
        m = dict(shared)
        m["qT"] = np.ascontiguousarray(q[b].T)
        m["kT"] = np.ascontiguousarray(k[b].T)
        m["vT"] = np.ascontiguousarray(v[b].T)
        in_maps.append(m)
    res = run_bass_kernel_spmd(nc, in_maps, list(range(B)), trace=trace, **kw)
    y = np.stack([res.results[b]["y"] for b in range(B)]).astype(np.float32)
    return y, res


def kernel(q, k, v, Wq, Wk, Wv, Wp, bp):
    y, _ = run({"q": q, "k": k, "v": v, "Wq": Wq, "Wk": Wk, "Wv": Wv,
                "Wp": Wp, "bp": bp})
    return y
